# revision 9
# baseline (speedup 1.0000x reference)
"""AttentionPool (single CLS query over ragged segments) on 8 TRN2 NeuronCores.

Per-core algorithm (whole segments sharded across cores, 8 segments/core):
  - Host folds the CLS query into the key projection:
        wq[i, h] = scale * sum_{j in head h} cls[j] * W_k[j, i]
    so scores[t, h] = sum_i embed[t, i] * wq[i, h].  The key bias b_k and the
    softmax max-subtraction shift scores by a per-(head, segment) constant
    that cancels in softmax => omitted (|s| <~ 25 << 88, exp can't overflow).
  - Host casts embed to bf16 (the matmuls are bf16 anyway), halving HBM
    traffic: ~40MB/core, the memory roofline this kernel chases.
  - Device, per 512-token block (4 chunks of 128 tokens):
      * one DMA of the natural-layout bf16 block (+ a ones column for the
        softmax denominator),
      * PE transposes each 128x128 tile to d-major (bf16 transpose-mode,
        PSUM->SBUF evacuation split across DVE/ACT),
      * scores via 4 column-tiled (128x32) concurrent matmuls, one PE tile
        per token chunk, accumulating 10 d-chunks into one PSUM bank,
      * ACT exp -> p (bf16, token chunks packed in 32-partition groups),
      * p transposed to token-major via 2x2 row-tiled (32x128) transposes,
      * num[h, d] += p^T @ x via 4 column-tiled concurrent matmuls; the ones
        column makes column 1280 the softmax denominator for free.
  - Segment end: denominators broadcast-summed across the 4 partition groups
    with one select-matrix matmul, reciprocal, mask the off-head d-slices,
    and 10 tiny matmuls contract the 128-partition group dim to finish
    out[i] = num[head(i), i] / denom[head(i)].

Self-contained: hardcodes the problem shapes; handles arbitrary cu_lens by
padding each segment slot to a fixed chunk grid (masked), degenerating to
zero overhead for the expected equal-length segmentation.
"""

import math

import numpy as np

H = 20        # heads
D = 1280      # embed dim
DH = D // H   # head dim (64)
P = 128       # partitions
G = 32        # partition group (PE tile column width)
NCORES = 8
BLK = 4       # chunks per block (512 tokens)
DC = D // P   # 10 d-chunks


def _ceil_div(a, b):
    return -(-a // b)


def _build_program(S, K, use_mask):
    """Build the SPMD Bass program: S segment slots x K chunks x 128 tokens."""
    import concourse.tile as tile
    from concourse import bacc, mybir
    from concourse.masks import make_identity

    f32 = mybir.dt.float32
    bf16 = mybir.dt.bfloat16
    Copy = mybir.ActivationFunctionType.Copy
    Exp = mybir.ActivationFunctionType.Exp

    Tpad = S * K * P
    nblk = _ceil_div(K, BLK)

    nc = bacc.Bacc()
    xin = nc.dram_tensor("xin", [Tpad, D], bf16, kind="ExternalInput")
    wqin = nc.dram_tensor("wqin", [D, H], bf16, kind="ExternalInput")
    selbin = nc.dram_tensor("selbin", [P, P], f32, kind="ExternalInput")
    dmaskin = nc.dram_tensor("dmaskin", [P, D], f32, kind="ExternalInput")
    idin = nc.dram_tensor("idin", [P, H], bf16, kind="ExternalInput")
    maskin = None
    if use_mask:
        maskin = nc.dram_tensor("maskin", [Tpad], f32, kind="ExternalInput")
    out = nc.dram_tensor("out", [S, D], f32, kind="ExternalOutput")

    with tile.TileContext(nc) as tc:
        with tc.tile_pool(name="persist", bufs=1) as persist:
            identity_bf = persist.tile([P, P], bf16)
            make_identity(nc, identity_bf)
            wq_sb = persist.tile([P, DC, H], bf16)
            nc.sync.dma_start(
                out=wq_sb, in_=wqin[:, :].rearrange("(dc p) h -> p dc h", p=P))
            selb_sb = persist.tile([P, P], f32)
            nc.sync.dma_start(out=selb_sb, in_=selbin[:, :])
            dmask_sb = persist.tile([P, D], f32)
            nc.sync.dma_start(out=dmask_sb, in_=dmaskin[:, :])
            id4_sb = persist.tile([P, H], bf16)
            nc.sync.dma_start(out=id4_sb, in_=idin[:, :])

            with tc.tile_pool(name="xn", bufs=3) as xn_pool, \
                 tc.tile_pool(name="xbt", bufs=2) as xbt_pool, \
                 tc.tile_pool(name="p4", bufs=2) as p4_pool, \
                 tc.tile_pool(name="pt", bufs=2) as pt_pool, \
                 tc.tile_pool(name="sm", bufs=2) as sm_pool, \
                 tc.tile_pool(name="msk", bufs=2) as msk_pool, \
                 tc.tile_pool(name="ps_tr", bufs=2, space="PSUM") as ps_tr, \
                 tc.tile_pool(name="ps_s4", bufs=1, space="PSUM") as ps_s4, \
                 tc.tile_pool(name="ps_pt", bufs=1, space="PSUM") as ps_pt, \
                 tc.tile_pool(name="ps_num", bufs=1, space="PSUM") as ps_num:
                for seg in range(S):
                    psum_num = ps_num.tile([P, D + 1], f32, tag="num")
                    if use_mask:
                        mask_sb = msk_pool.tile([P, K], f32, tag="msk")
                        nc.sync.dma_start(
                            out=mask_sb,
                            in_=maskin[seg * K * P:(seg + 1) * K * P]
                            .rearrange("(k p) -> p k", p=P))
                    if K < BLK:
                        # some PE column tiles never run: clear their rows
                        nc.vector.memset(psum_num, 0.0)
                    for blk in range(nblk):
                        bc = min(BLK, K - blk * BLK)
                        tok0 = seg * K * P + blk * BLK * P

                        xn = xn_pool.tile([P, BLK, D + 1], bf16, tag="xn")
                        nc.sync.dma_start(
                            out=xn[:, :bc, 0:D],
                            in_=xin[tok0:tok0 + bc * P, :].rearrange(
                                "(k p) i -> p k i", p=P))
                        nc.vector.memset(xn[:, :bc, D:D + 1], 1.0)

                        # d-major tiles via PE transpose-mode
                        xbt = xbt_pool.tile([P, BLK, DC, P], bf16, tag="xbt")
                        for k in range(bc):
                            for half in range(2):
                                tr = ps_tr.tile([P, 5 * P], bf16, tag="tr")
                                for j5 in range(5):
                                    dc = half * 5 + j5
                                    nc.tensor.transpose(
                                        tr[:, j5 * P:(j5 + 1) * P],
                                        xn[:, k, dc * P:(dc + 1) * P],
                                        identity_bf)
                                dst = xbt[:, k, half * 5:half * 5 + 5, :]
                                src = tr.rearrange("p (j t) -> p j t", j=5)
                                if (k + half) % 2:
                                    nc.scalar.copy(out=dst, in_=src)
                                else:
                                    nc.vector.tensor_copy(out=dst, in_=src)

                        # scores: 4 column-tiled concurrent matmuls (one per
                        # token chunk), 10 accumulating d-chunk rounds
                        s4 = ps_s4.tile([P, P], f32, tag="s4")
                        for dc in range(DC):
                            for j in range(bc):
                                nc.tensor.matmul(
                                    s4[G * j:G * j + H, :],
                                    lhsT=wq_sb[:, dc, :],
                                    rhs=xbt[:, j, dc, :],
                                    start=(dc == 0), stop=(dc == DC - 1),
                                    skip_group_check=True,
                                    tile_position=(0, G * j))

                        p4 = p4_pool.tile([P, P], bf16, tag="p4")
                        nc.scalar.activation(out=p4, in_=s4, func=Exp)

                        # p -> token-major, 2x2 row-tiled transposes
                        pt = pt_pool.tile([P, BLK, G], bf16, tag="pt")
                        nc.vector.memset(pt, 0.0)
                        for pair in range((bc + 1) // 2):
                            ptps = ps_pt.tile([P, 2, 1024], bf16, tag="ptps")
                            for jj in range(2):
                                j = pair * 2 + jj
                                if j >= bc:
                                    continue
                                nc.tensor.transpose(
                                    ptps[:, jj, 0:H],
                                    p4[G * j:G * j + H, :],
                                    id4_sb[G * j:G * j + H, :],
                                    tile_position=(G * j, 0))
                            nj = min(2, bc - pair * 2)
                            nc.vector.tensor_copy(
                                out=pt[:, pair * 2:pair * 2 + nj, 0:H],
                                in_=ptps[:, 0:nj, 0:H])
                        if use_mask:
                            for k in range(bc):
                                nc.vector.tensor_scalar_mul(
                                    pt[:, k, 0:H], pt[:, k, 0:H],
                                    mask_sb[:, blk * BLK + k:blk * BLK + k + 1])

                        # num (+ denominator column): 4 column-tiled
                        # concurrent matmuls, one per token chunk
                        last_bc = K - (nblk - 1) * BLK
                        for n0, n1 in ((0, 512), (512, 1024), (1024, D + 1)):
                            for j in range(bc):
                                is_last = (blk == nblk - 1) or (
                                    blk == nblk - 2 and j >= last_bc)
                                nc.tensor.matmul(
                                    psum_num[G * j:G * (j + 1), n0:n1],
                                    lhsT=pt[:, j, :],
                                    rhs=xn[:, j, n0:n1],
                                    start=(blk == 0), stop=is_last,
                                    skip_group_check=True,
                                    tile_position=(0, G * j))

                    # ---- segment tail
                    dsum = sm_pool.tile([P, 1], f32, tag="dsum")
                    nc.scalar.activation(
                        out=dsum, in_=psum_num[:, D:D + 1], func=Copy,
                        bias=1e-30)
                    psd = ps_s4.tile([P, 1], f32, tag="s4")
                    nc.tensor.matmul(
                        psd, lhsT=selb_sb, rhs=dsum, start=True, stop=True,
                        skip_group_check=True)
                    recf = sm_pool.tile([P, 1], f32, tag="recf")
                    nc.vector.reciprocal(out=recf, in_=psd)
                    masked = sm_pool.tile([P, D], f32, tag="masked")
                    nc.vector.tensor_mul(masked, psum_num[:, 0:D], dmask_sb)
                    pso = ps_pt.tile([P, DC], f32, tag="ptps")
                    for ic in range(DC):
                        nc.tensor.matmul(
                            pso[:, ic:ic + 1],
                            lhsT=masked[:, ic * P:(ic + 1) * P],
                            rhs=recf,
                            start=True, stop=True, skip_group_check=True)
                    outv = sm_pool.tile([P, DC], f32, tag="outv")
                    nc.vector.tensor_copy(out=outv, in_=pso)
                    nc.gpsimd.dma_start(
                        out=out[seg:seg + 1, :].rearrange(
                            "s (f p) -> (s p) f", p=P),
                        in_=outv)
    nc.finalize()
    return nc


def _host_consts():
    """selB, dmask, id4 device constants."""
    selb = np.zeros((P, P), dtype=np.float32)
    for pq in range(P):
        for q in range(P):
            if pq % G == q % G:
                selb[pq, q] = 1.0
    dmask = np.zeros((P, D), dtype=np.float32)
    for j in range(4):
        for h in range(H):
            dmask[G * j + h, DH * h:DH * (h + 1)] = 1.0
    id4 = np.zeros((P, H), dtype=np.float32)
    for j in range(4):
        for a in range(H):
            id4[G * j + a, a] = 1.0
    return selb, dmask, id4


def _plan(cu_lens):
    """Host-side sharding plan from cu_lens. Returns (S, K, assignments,
    use_mask) where assignments[core] = list of (slot, seg_idx, start, end)."""
    cu = [int(v) for v in cu_lens]
    n = len(cu) - 1
    lens = [cu[i + 1] - cu[i] for i in range(n)]
    S = _ceil_div(n, NCORES)
    max_len = max(lens) if lens else 1
    K = max(1, _ceil_div(max_len, P))
    use_mask = (n != S * NCORES) or any(l != K * P for l in lens)
    assignments = []
    for i in range(NCORES):
        rows = []
        for s in range(S):
            seg = i * S + s
            if seg < n:
                rows.append((s, seg, cu[seg], cu[seg + 1]))
        assignments.append(rows)
    return S, K, assignments, use_mask


def prepare(cls, embed, cu_lens, W_k):
    """Host-side: plan the sharding, fold cls into W_k, cast to bf16, build
    the Bass program and per-core input maps."""
    import ml_dtypes

    bf16 = ml_dtypes.bfloat16
    cls = np.asarray(cls, dtype=np.float32).reshape(-1)
    embed = np.asarray(embed, dtype=np.float32)
    W_k = np.ascontiguousarray(np.asarray(W_k, dtype=np.float32))
    cu = np.asarray(cu_lens).astype(np.int64)
    n = cu.shape[0] - 1

    scale = np.float32(1.0 / math.sqrt(DH))
    # wq[i, h] = scale * sum_{j in head h} cls[j] * W_k[j, i]
    wq = np.einsum("hji,hj->ih", W_k.reshape(H, DH, D),
                   cls.reshape(H, DH)).astype(np.float32) * scale
    wq_b = wq.astype(bf16)

    embed_b = embed.astype(bf16)

    S, K, assignments, use_mask = _plan(cu)
    Tpad = S * K * P
    nc = _build_program(S, K, use_mask)

    selb, dmask, id4 = _host_consts()
    id4_b = id4.astype(bf16)

    in_maps = []
    for i in range(NCORES):
        rows = assignments[i]
        contiguous = (
            not use_mask
            and len(rows) == S
            and all(end - start == K * P for (_s, _seg, start, end) in rows)
            and all(rows[j][3] == rows[j + 1][2] for j in range(len(rows) - 1))
        )
        m = {"wqin": wq_b, "selbin": selb, "dmaskin": dmask, "idin": id4_b}
        if contiguous:
            m["xin"] = embed_b[rows[0][2]:rows[-1][3]]
        else:
            shard = np.zeros((Tpad, D), dtype=bf16)
            mask = np.zeros((Tpad,), dtype=np.float32)
            for (s, _seg, start, end) in rows:
                L = end - start
                shard[s * K * P:s * K * P + L] = embed_b[start:end]
                mask[s * K * P:s * K * P + L] = 1.0
            m["xin"] = shard
            if use_mask:
                m["maskin"] = mask
        in_maps.append(m)
    return nc, in_maps, assignments, n


def gather(results, assignments, n):
    full = np.zeros((n, 1, D), dtype=np.float32)
    for i in range(NCORES):
        core_out = np.asarray(results[i]["out"])
        for (s, seg, _start, _end) in assignments[i]:
            full[seg, 0, :] = core_out[s]
    return full


def kernel(cls, embed, cu_lens, max_len, W_k, b_k):
    from concourse.bass_utils import run_bass_kernel_spmd

    nc, in_maps, assignments, n = prepare(cls, embed, cu_lens, W_k)
    res = run_bass_kernel_spmd(nc, in_maps, core_ids=list(range(NCORES)))
    return gather(res.results, assignments, n)


# revision 11
# speedup vs baseline: 1.0092x; 1.0092x over previous
"""AttentionPool (single CLS query over ragged segments) on 8 TRN2 NeuronCores.

Per-core algorithm (whole segments sharded across cores, 8 segments/core):
  - Host folds the CLS query into the key projection:
        wq[i, h] = scale * sum_{j in head h} cls[j] * W_k[j, i]
    so scores[t, h] = sum_i embed[t, i] * wq[i, h].  The key bias b_k and the
    softmax max-subtraction shift scores by a per-(head, segment) constant
    that cancels in softmax => omitted (|s| <~ 25 << 88, exp can't overflow).
  - Host casts embed to bf16 (the matmuls are bf16 anyway), halving HBM
    traffic: ~40MB/core, the memory roofline this kernel chases.
  - Device, per 512-token block (4 chunks of 128 tokens):
      * one DMA of the natural-layout bf16 block (+ a ones column for the
        softmax denominator),
      * PE transposes each 128x128 tile to d-major (bf16 transpose-mode,
        PSUM->SBUF evacuation split across DVE/ACT),
      * scores via 4 column-tiled (128x32) concurrent matmuls, one PE tile
        per token chunk, accumulating 10 d-chunks into one PSUM bank,
      * ACT exp -> p (bf16, token chunks packed in 32-partition groups),
      * p transposed to token-major via 2x2 row-tiled (32x128) transposes,
      * num[h, d] += p^T @ x via 4 column-tiled concurrent matmuls; the ones
        column makes column 1280 the softmax denominator for free.
  - Segment end: denominators broadcast-summed across the 4 partition groups
    with one select-matrix matmul, reciprocal, mask the off-head d-slices,
    and 10 tiny matmuls contract the 128-partition group dim to finish
    out[i] = num[head(i), i] / denom[head(i)].

Self-contained: hardcodes the problem shapes; handles arbitrary cu_lens by
padding each segment slot to a fixed chunk grid (masked), degenerating to
zero overhead for the expected equal-length segmentation.
"""

import math

import numpy as np

H = 20        # heads
D = 1280      # embed dim
DH = D // H   # head dim (64)
P = 128       # partitions
G = 32        # partition group (PE tile column width)
NCORES = 8
BLK = 4       # chunks per block (512 tokens)
DC = D // P   # 10 d-chunks


def _ceil_div(a, b):
    return -(-a // b)


def _build_program(S, K, use_mask):
    """Build the SPMD Bass program: S segment slots x K chunks x 128 tokens."""
    import concourse.tile as tile
    from concourse import bacc, mybir
    from concourse.masks import make_identity

    f32 = mybir.dt.float32
    bf16 = mybir.dt.bfloat16
    Copy = mybir.ActivationFunctionType.Copy
    Exp = mybir.ActivationFunctionType.Exp

    Tpad = S * K * P
    nblk = _ceil_div(K, BLK)

    nc = bacc.Bacc()
    xin = nc.dram_tensor("xin", [Tpad, D], bf16, kind="ExternalInput")
    wqin = nc.dram_tensor("wqin", [D, H], bf16, kind="ExternalInput")
    selbin = nc.dram_tensor("selbin", [P, P], f32, kind="ExternalInput")
    dmaskin = nc.dram_tensor("dmaskin", [P, D], f32, kind="ExternalInput")
    idin = nc.dram_tensor("idin", [P, H], bf16, kind="ExternalInput")
    maskin = None
    if use_mask:
        maskin = nc.dram_tensor("maskin", [Tpad], f32, kind="ExternalInput")
    out = nc.dram_tensor("out", [S, D], f32, kind="ExternalOutput")

    with tile.TileContext(nc) as tc:
        with tc.tile_pool(name="persist", bufs=1) as persist:
            identity_bf = persist.tile([P, P], bf16)
            make_identity(nc, identity_bf)
            wq_sb = persist.tile([P, DC, H], bf16)
            nc.sync.dma_start(
                out=wq_sb, in_=wqin[:, :].rearrange("(dc p) h -> p dc h", p=P))
            selb_sb = persist.tile([P, P], f32)
            nc.sync.dma_start(out=selb_sb, in_=selbin[:, :])
            dmask_sb = persist.tile([P, D], f32)
            nc.sync.dma_start(out=dmask_sb, in_=dmaskin[:, :])
            id4_sb = persist.tile([P, H], bf16)
            nc.sync.dma_start(out=id4_sb, in_=idin[:, :])

            with tc.tile_pool(name="xn", bufs=6) as xn_pool, \
                 tc.tile_pool(name="xbt", bufs=2) as xbt_pool, \
                 tc.tile_pool(name="p4", bufs=2) as p4_pool, \
                 tc.tile_pool(name="pt", bufs=2) as pt_pool, \
                 tc.tile_pool(name="sm", bufs=2) as sm_pool, \
                 tc.tile_pool(name="msk", bufs=2) as msk_pool, \
                 tc.tile_pool(name="ps_tr", bufs=2, space="PSUM") as ps_tr, \
                 tc.tile_pool(name="ps_s4", bufs=1, space="PSUM") as ps_s4, \
                 tc.tile_pool(name="ps_pt", bufs=1, space="PSUM") as ps_pt, \
                 tc.tile_pool(name="ps_num", bufs=1, space="PSUM") as ps_num:
                Kg = _ceil_div(K, 4)       # chunks per PE column group
                ngrp = _ceil_div(K, Kg)    # column groups (<= 4)
                for seg in range(S):
                    psum_num = ps_num.tile([P, D + 1], f32, tag="num")
                    if use_mask:
                        mask_sb = msk_pool.tile([P, K], f32, tag="msk")
                        nc.sync.dma_start(
                            out=mask_sb,
                            in_=maskin[seg * K * P:(seg + 1) * K * P]
                            .rearrange("(k p) -> p k", p=P))
                    if ngrp < 4:
                        # some PE column tiles never run: clear their rows
                        nc.vector.memset(psum_num, 0.0)

                    # phase 1: stream blocks in, transpose to d-major
                    xns = []
                    xbt = xbt_pool.tile([P, K, DC, P], bf16, tag="xbt")
                    for blk in range(nblk):
                        bc = min(BLK, K - blk * BLK)
                        tok0 = seg * K * P + blk * BLK * P
                        xn = xn_pool.tile([P, BLK, D + 1], bf16, tag="xn")
                        xns.append(xn)
                        nc.sync.dma_start(
                            out=xn[:, :bc, 0:D],
                            in_=xin[tok0:tok0 + bc * P, :].rearrange(
                                "(k p) i -> p k i", p=P))
                        nc.vector.memset(xn[:, :bc, D:D + 1], 1.0)
                        for k in range(bc):
                            for half in range(2):
                                tr = ps_tr.tile([P, 5 * P], bf16, tag="tr")
                                for j5 in range(5):
                                    dc = half * 5 + j5
                                    nc.tensor.transpose(
                                        tr[:, j5 * P:(j5 + 1) * P],
                                        xn[:, k, dc * P:(dc + 1) * P],
                                        identity_bf)
                                dst = xbt[:, blk * BLK + k,
                                          half * 5:half * 5 + 5, :]
                                src = tr.rearrange("p (j t) -> p j t", j=5)
                                if (k + half) % 2:
                                    nc.scalar.copy(out=dst, in_=src)
                                else:
                                    nc.vector.tensor_copy(out=dst, in_=src)

                    # phase 2: scores for the whole segment, 4 column-tiled
                    # concurrent matmuls x 10 accumulating d-chunk rounds,
                    # each streaming up to 512 tokens (group j = chunks
                    # [j*Kg, j*Kg+gc))
                    gcs = [min(Kg, K - j * Kg) for j in range(ngrp)]
                    s4 = ps_s4.tile([P, Kg * P], f32, tag="s4")
                    for dc in range(DC):
                        for j in range(ngrp):
                            gc = gcs[j]
                            nc.tensor.matmul(
                                s4[G * j:G * j + H, 0:gc * P],
                                lhsT=wq_sb[:, dc, :],
                                rhs=xbt[:, j * Kg:j * Kg + gc, dc, :],
                                start=(dc == 0), stop=(dc == DC - 1),
                                skip_group_check=True,
                                tile_position=(0, G * j))

                    # phase 3: exp
                    p4 = p4_pool.tile([P, Kg * P], bf16, tag="p4")
                    nc.scalar.activation(out=p4, in_=s4, func=Exp)

                    # phase 4: p -> token-major, row-tiled transpose pairs
                    pt = pt_pool.tile([P, K, G], bf16, tag="pt")
                    nc.vector.memset(pt, 0.0)
                    pend = []
                    for j in range(ngrp):
                        for ci in range(gcs[j]):
                            pend.append((j, ci))
                    for pair0 in range(0, len(pend), 2):
                        chunk_pair = pend[pair0:pair0 + 2]
                        ptps = ps_pt.tile([P, 2, 1024], bf16, tag="ptps")
                        for jj, (j, ci) in enumerate(chunk_pair):
                            nc.tensor.transpose(
                                ptps[:, jj, 0:H],
                                p4[G * j:G * j + H, ci * P:(ci + 1) * P],
                                id4_sb[G * j:G * j + H, :],
                                tile_position=(G * j, 0))
                        for jj, (j, ci) in enumerate(chunk_pair):
                            c = j * Kg + ci
                            nc.vector.tensor_copy(
                                out=pt[:, c, 0:H], in_=ptps[:, jj, 0:H])
                    if use_mask:
                        for c in range(K):
                            nc.vector.tensor_scalar_mul(
                                pt[:, c, 0:H], pt[:, c, 0:H],
                                mask_sb[:, c:c + 1])

                    # phase 5: num (+ denominator column), 4 column-tiled
                    # concurrent matmuls per round
                    for n0, n1 in ((0, 512), (512, 1024), (1024, D + 1)):
                        for ci in range(Kg):
                            for j in range(ngrp):
                                if ci >= gcs[j]:
                                    continue
                                c = j * Kg + ci
                                nc.tensor.matmul(
                                    psum_num[G * j:G * (j + 1), n0:n1],
                                    lhsT=pt[:, c, :],
                                    rhs=xns[c // BLK][:, c % BLK, n0:n1],
                                    start=(ci == 0), stop=(ci == gcs[j] - 1),
                                    skip_group_check=True,
                                    tile_position=(0, G * j))

                    # ---- segment tail
                    dsum = sm_pool.tile([P, 1], f32, tag="dsum")
                    nc.scalar.activation(
                        out=dsum, in_=psum_num[:, D:D + 1], func=Copy,
                        bias=1e-30)
                    psd = ps_s4.tile([P, 1], f32, tag="s4")
                    nc.tensor.matmul(
                        psd, lhsT=selb_sb, rhs=dsum, start=True, stop=True,
                        skip_group_check=True)
                    recf = sm_pool.tile([P, 1], f32, tag="recf")
                    nc.vector.reciprocal(out=recf, in_=psd)
                    masked = sm_pool.tile([P, D], f32, tag="masked")
                    nc.vector.tensor_mul(masked, psum_num[:, 0:D], dmask_sb)
                    pso = ps_pt.tile([P, DC], f32, tag="ptps")
                    for ic in range(DC):
                        nc.tensor.matmul(
                            pso[:, ic:ic + 1],
                            lhsT=masked[:, ic * P:(ic + 1) * P],
                            rhs=recf,
                            start=True, stop=True, skip_group_check=True)
                    outv = sm_pool.tile([P, DC], f32, tag="outv")
                    nc.vector.tensor_copy(out=outv, in_=pso)
                    nc.gpsimd.dma_start(
                        out=out[seg:seg + 1, :].rearrange(
                            "s (f p) -> (s p) f", p=P),
                        in_=outv)
    nc.finalize()
    return nc


def _host_consts():
    """selB, dmask, id4 device constants."""
    selb = np.zeros((P, P), dtype=np.float32)
    for pq in range(P):
        for q in range(P):
            if pq % G == q % G:
                selb[pq, q] = 1.0
    dmask = np.zeros((P, D), dtype=np.float32)
    for j in range(4):
        for h in range(H):
            dmask[G * j + h, DH * h:DH * (h + 1)] = 1.0
    id4 = np.zeros((P, H), dtype=np.float32)
    for j in range(4):
        for a in range(H):
            id4[G * j + a, a] = 1.0
    return selb, dmask, id4


def _plan(cu_lens):
    """Host-side sharding plan from cu_lens. Returns (S, K, assignments,
    use_mask) where assignments[core] = list of (slot, seg_idx, start, end)."""
    cu = [int(v) for v in cu_lens]
    n = len(cu) - 1
    lens = [cu[i + 1] - cu[i] for i in range(n)]
    S = _ceil_div(n, NCORES)
    max_len = max(lens) if lens else 1
    K = max(1, _ceil_div(max_len, P))
    use_mask = (n != S * NCORES) or any(l != K * P for l in lens)
    assignments = []
    for i in range(NCORES):
        rows = []
        for s in range(S):
            seg = i * S + s
            if seg < n:
                rows.append((s, seg, cu[seg], cu[seg + 1]))
        assignments.append(rows)
    return S, K, assignments, use_mask


def prepare(cls, embed, cu_lens, W_k):
    """Host-side: plan the sharding, fold cls into W_k, cast to bf16, build
    the Bass program and per-core input maps."""
    import ml_dtypes

    bf16 = ml_dtypes.bfloat16
    cls = np.asarray(cls, dtype=np.float32).reshape(-1)
    embed = np.asarray(embed, dtype=np.float32)
    W_k = np.ascontiguousarray(np.asarray(W_k, dtype=np.float32))
    cu = np.asarray(cu_lens).astype(np.int64)
    n = cu.shape[0] - 1

    scale = np.float32(1.0 / math.sqrt(DH))
    # wq[i, h] = scale * sum_{j in head h} cls[j] * W_k[j, i]
    wq = np.einsum("hji,hj->ih", W_k.reshape(H, DH, D),
                   cls.reshape(H, DH)).astype(np.float32) * scale
    wq_b = wq.astype(bf16)

    embed_b = embed.astype(bf16)

    S, K, assignments, use_mask = _plan(cu)
    Tpad = S * K * P
    nc = _build_program(S, K, use_mask)

    selb, dmask, id4 = _host_consts()
    id4_b = id4.astype(bf16)

    in_maps = []
    for i in range(NCORES):
        rows = assignments[i]
        contiguous = (
            not use_mask
            and len(rows) == S
            and all(end - start == K * P for (_s, _seg, start, end) in rows)
            and all(rows[j][3] == rows[j + 1][2] for j in range(len(rows) - 1))
        )
        m = {"wqin": wq_b, "selbin": selb, "dmaskin": dmask, "idin": id4_b}
        if contiguous:
            m["xin"] = embed_b[rows[0][2]:rows[-1][3]]
        else:
            shard = np.zeros((Tpad, D), dtype=bf16)
            mask = np.zeros((Tpad,), dtype=np.float32)
            for (s, _seg, start, end) in rows:
                L = end - start
                shard[s * K * P:s * K * P + L] = embed_b[start:end]
                mask[s * K * P:s * K * P + L] = 1.0
            m["xin"] = shard
            if use_mask:
                m["maskin"] = mask
        in_maps.append(m)
    return nc, in_maps, assignments, n


def gather(results, assignments, n):
    full = np.zeros((n, 1, D), dtype=np.float32)
    for i in range(NCORES):
        core_out = np.asarray(results[i]["out"])
        for (s, seg, _start, _end) in assignments[i]:
            full[seg, 0, :] = core_out[s]
    return full


def kernel(cls, embed, cu_lens, max_len, W_k, b_k):
    from concourse.bass_utils import run_bass_kernel_spmd

    nc, in_maps, assignments, n = prepare(cls, embed, cu_lens, W_k)
    res = run_bass_kernel_spmd(nc, in_maps, core_ids=list(range(NCORES)))
    return gather(res.results, assignments, n)
